# revision 22
# baseline (speedup 1.0000x reference)
"""Region-augmented embedding lookup (MeanEncoder) on 8 TRN2 NeuronCores.

Reference computation (per batch b, position l):
    out[b,l,0,:] = tanh( sum_{j=0..6} W[ seq_pad[b, l+j]*7 + j , :] ) * (seq[b,l]!=0)

Strategy: data parallel, W replicated (cast to bf16 on host), each core
takes 2 of 16 sequences.

Device kernel, per super-group of up to 8 tiles (tile = 122 output
positions from 128 gathered window positions):
  1. One indirect DMA per tile gathers 128 per-token contiguous 7x128
     bf16 blocks W[tok*7 : tok*7+7, :] into a slice of a [128, 8*896]
     SBUF tile (descriptor = 1792B). The TRN2 indirect DMA consumes
     exactly ONE index per dest partition and streams the whole
     per-partition dest from that base (multi-column offset APs
     silently use only the first index), so per-tile instructions are
     forced. SWDGE emission is ~994ns fixed + 0.34ns/descriptor per
     instruction -> the 34-instruction gather stream (~1.41us cadence,
     ~48us) is the kernel's hard floor. bf16 (host-cast W) halves the
     HBM traffic so the SDMA drain hides entirely under that emission.
  2. Shifted region-sum out[i] = sum_j G[i+j, seg_j] on the tensor
     engine, per half-group of 4 tiles: 7 bf16 matmuls, lhsT =
     zero-padded identity slice idp[:, j:j+128] (128 weight cols),
     rhs = the j-th 128-col segment of 4 tiles (N=512), PSUM fp32
     accumulate. Exact given bf16 inputs. (A DVE shifted-add is
     illegal: the BIR verifier requires partition starts to be
     32-aligned and >32-partition accesses to start at partition 0,
     so cross-partition shifts can only be done by the PE.)
  3. One scalar-engine tanh per half ([122, 512] at once), bf16 out.
  4. One contiguous store per group: out_dram[:, c0*128:(c0+ng)*128],
     2KB-per-partition descriptors (vs 512B interleaved rows at
     baseline, which serialized 63us of store FIFO time).
Output DRAM layout is [122, NTILES*128] bf16 in tile-column order; the
host de-permutes tiles, applies the (seq!=0) mask, and casts to fp32.
Out-of-sequence window positions use token id 0 (= the reference pad).
Numerics: only error sources are the bf16 cast of W and the bf16
output store; max rel err ~3.8e-3 vs the fp32 reference (gate 2e-2).
"""

import numpy as np
import ml_dtypes

import concourse.bass as bass
import concourse.tile as tile
from concourse import bacc, mybir
from concourse.bass_utils import run_bass_kernel_spmd

VOCAB = 50000
EMB = 128
RADIUS = 3
REGION = 7
B, L, C = 16, 2048, 1
NCORES = 8
SEQ_PER_CORE = B // NCORES           # 2
P = 128                              # gathered window positions per tile
TOUT = P - (REGION - 1)              # 122 output positions per tile
TILES_PER_SEQ = -(-L // TOUT)        # 17 (16*122=1952, last tile 96 rows)
NTILES = SEQ_PER_CORE * TILES_PER_SEQ  # 34
BLK = REGION * EMB                   # 896 elements per gathered block
GRP = 8                              # tiles per merged gather group
HALF = 4                             # tiles per compute half (N=512)

# tile-column order: all full tiles first, the two ragged 17th tiles last
TILE_ORDER = [(s, k) for s in range(SEQ_PER_CORE)
              for k in range(TILES_PER_SEQ - 1)]
TILE_ORDER += [(s, TILES_PER_SEQ - 1) for s in range(SEQ_PER_CORE)]
# groups of consecutive columns: four full groups of 8, ragged pair
# last (small: short compute chain after the final gather)
GROUPS = [(0, 8), (8, 8), (16, 8), (24, 8), (32, 2)]


def _build_nc():
    nc = bacc.Bacc("TRN2", target_bir_lowering=False, debug=False,
                   enable_partition_id=False)

    w = nc.declare_dram_parameter("w", [VOCAB * REGION, EMB], mybir.dt.bfloat16, isOutput=False)
    gidx = nc.declare_dram_parameter("gidx", [P, NTILES], mybir.dt.int32, isOutput=False)
    ident = nc.declare_dram_parameter("ident", [P, P + REGION - 1], mybir.dt.bfloat16, isOutput=False)
    out = nc.declare_dram_parameter("out", [TOUT, NTILES * EMB], mybir.dt.bfloat16, isOutput=True)

    from contextlib import ExitStack
    with tile.TileContext(nc) as tc, ExitStack() as ctx:
        const_pool = ctx.enter_context(tc.tile_pool(name="const", bufs=1))
        gpool = ctx.enter_context(tc.tile_pool(name="gather", bufs=4))
        ppool = ctx.enter_context(tc.tile_pool(name="psum", bufs=6, space="PSUM"))
        opool = ctx.enter_context(tc.tile_pool(name="out", bufs=3))

        gidx_sb = const_pool.tile([P, NTILES], mybir.dt.int32)
        idp_sb = const_pool.tile([P, P + REGION - 1], mybir.dt.bfloat16)
        nc.sync.dma_start(gidx_sb[:], gidx.ap())
        nc.scalar.dma_start(idp_sb[:], ident.ap())

        for c0, ng in GROUPS:
            gsb = gpool.tile([P, GRP * BLK], mybir.dt.bfloat16, tag="g")
            gv = gsb[:].rearrange("p (u j e) -> p u j e", u=GRP, j=REGION)
            # one indirect DMA per tile: the HW consumes exactly one index
            # per dest partition and streams 1792B from that base
            for u in range(ng):
                nc.gpsimd.indirect_dma_start(
                    out=gsb[:, u * BLK: (u + 1) * BLK],
                    out_offset=None,
                    in_=w.ap(),
                    in_offset=bass.IndirectOffsetOnAxis(
                        ap=gidx_sb[:, c0 + u: c0 + u + 1], axis=0),
                )
            for h0 in range(0, ng, HALF):
                nu = min(HALF, ng - h0)
                psum = ppool.tile([P, HALF * EMB], mybir.dt.float32, tag="ps")
                for j in range(REGION):
                    nc.tensor.matmul(
                        out=psum[:, : nu * EMB],
                        lhsT=idp_sb[:, j: j + P],
                        rhs=gv[:, h0: h0 + nu, j, :],
                        start=(j == 0),
                        stop=(j == REGION - 1),
                    )
                o = opool.tile([TOUT, HALF * EMB], mybir.dt.bfloat16, tag="o")
                nc.scalar.activation(
                    o[:, : nu * EMB], psum[:TOUT, : nu * EMB],
                    mybir.ActivationFunctionType.Tanh,
                )
                # per-half store: 1KB-per-partition contiguous descriptors,
                # and the final store (-> teardown receipt) fires earlier
                nc.sync.dma_start(
                    out.ap()[:, (c0 + h0) * EMB: (c0 + h0 + nu) * EMB],
                    o[:, : nu * EMB])
    nc.compile()
    return nc


def _host_prep(seq, W):
    s = seq.reshape(B, L)
    w_bf16 = np.ascontiguousarray(W.astype(ml_dtypes.bfloat16))
    ident = np.zeros((P, P + REGION - 1), ml_dtypes.bfloat16)
    ident[np.arange(P), np.arange(P)] = 1

    in_maps = []
    for c in range(NCORES):
        gidx_r = np.zeros((P, NTILES), np.int32)
        for col, (sq, k) in enumerate(TILE_ORDER):
            b = c * SEQ_PER_CORE + sq
            v = k * TOUT - RADIUS + np.arange(P)
            tok = np.where((v >= 0) & (v < L), s[b, np.clip(v, 0, L - 1)], 0)
            gidx_r[:, col] = tok.astype(np.int32) * REGION
        in_maps.append({
            "w": w_bf16,
            "gidx": gidx_r,
            "ident": ident,
        })
    return in_maps


_NC_CACHE = None


def run(seq, W, trace=False, **spmd_kwargs):
    global _NC_CACHE
    if _NC_CACHE is None:
        _NC_CACHE = _build_nc()
    nc = _NC_CACHE
    seq = np.asarray(seq)
    in_maps = _host_prep(seq, np.asarray(W))
    res = run_bass_kernel_spmd(
        nc, in_maps, core_ids=list(range(NCORES)), trace=trace, **spmd_kwargs
    )
    full = np.empty((B, L, EMB), np.float32)
    for c in range(NCORES):
        r = np.asarray(res.results[c]["out"]).astype(np.float32)
        r = r.reshape(TOUT, NTILES, EMB)
        for col, (sq, k) in enumerate(TILE_ORDER):
            b = c * SEQ_PER_CORE + sq
            q0 = k * TOUT
            nrows = min(TOUT, L - q0)
            full[b, q0: q0 + nrows] = r[:nrows, col]
    mask = (seq.reshape(B, L) != 0).astype(np.float32)
    full *= mask[:, :, None]
    return full[:, :, None, :], res


def kernel(seq, W):
    out, _ = run(np.asarray(seq), np.asarray(W))
    return out


# revision 23
# speedup vs baseline: 1.1728x; 1.1728x over previous
"""Region-augmented embedding lookup (MeanEncoder) on 8 TRN2 NeuronCores.

Reference computation (per batch b, position l):
    out[b,l,0,:] = tanh( sum_{j=0..6} W[ seq_pad[b, l+j]*7 + j , :] ) * (seq[b,l]!=0)

Strategy: data parallel, W replicated (cast to bf16 on host), each core
takes 2 of 16 sequences.

Device kernel, per super-group of up to 8 tiles (tile = 122 output
positions from 128 gathered window positions):
  1. One indirect DMA per tile gathers 128 per-token contiguous 7x128
     bf16 blocks W[tok*7 : tok*7+7, :] into a slice of a [128, 8*896]
     SBUF tile (descriptor = 1792B). The TRN2 indirect DMA consumes
     exactly ONE index per dest partition and streams the whole
     per-partition dest from that base (multi-column offset APs
     silently use only the first index), so per-tile instructions are
     forced. SWDGE emission is ~994ns fixed + 0.34ns/descriptor per
     instruction -> the 34-instruction gather stream (~1.41us cadence,
     ~48us) is the kernel's hard floor. bf16 (host-cast W) halves the
     HBM traffic so the SDMA drain hides entirely under that emission.
  2. Shifted region-sum out[i] = sum_j G[i+j, seg_j] on the tensor
     engine, per half-group of 4 tiles: 7 bf16 matmuls, lhsT =
     zero-padded identity slice idp[:, j:j+128] (128 weight cols),
     rhs = the j-th 128-col segment of 4 tiles (N=512), PSUM fp32
     accumulate. Exact given bf16 inputs. (A DVE shifted-add is
     illegal: the BIR verifier requires partition starts to be
     32-aligned and >32-partition accesses to start at partition 0,
     so cross-partition shifts can only be done by the PE.)
  3. One scalar-engine tanh per half ([122, 512] at once), bf16 out.
  4. One contiguous store per group: out_dram[:, c0*128:(c0+ng)*128],
     2KB-per-partition descriptors (vs 512B interleaved rows at
     baseline, which serialized 63us of store FIFO time).
Output DRAM layout is [122, NTILES*128] bf16 in tile-column order; the
host de-permutes tiles, applies the (seq!=0) mask, and casts to fp32.
Out-of-sequence window positions use token id 0 (= the reference pad).
Numerics: only error sources are the bf16 cast of W and the bf16
output store; max rel err ~3.8e-3 vs the fp32 reference (gate 2e-2).
"""

import numpy as np
import ml_dtypes

import concourse.bass as bass
import concourse.tile as tile
from concourse import bacc, mybir
from concourse.bass_utils import run_bass_kernel_spmd

VOCAB = 50000
EMB = 128
RADIUS = 3
REGION = 7
B, L, C = 16, 2048, 1
NCORES = 8
SEQ_PER_CORE = B // NCORES           # 2
P = 128                              # gathered window positions per tile
TOUT = P - (REGION - 1)              # 122 output positions per tile
TILES_PER_SEQ = -(-L // TOUT)        # 17 (16*122=1952, last tile 96 rows)
NTILES = SEQ_PER_CORE * TILES_PER_SEQ  # 34
BLK = REGION * EMB                   # 896 elements per gathered block
GRP = 8                              # tiles per merged gather group
HALF = 4                             # tiles per compute half (N=512)

# tile-column order: all full tiles first, the two ragged 17th tiles last
TILE_ORDER = [(s, k) for s in range(SEQ_PER_CORE)
              for k in range(TILES_PER_SEQ - 1)]
TILE_ORDER += [(s, TILES_PER_SEQ - 1) for s in range(SEQ_PER_CORE)]
# groups of consecutive columns: four full groups of 8, ragged pair
# last (small: short compute chain after the final gather)
GROUPS = [(0, 8), (8, 8), (16, 8), (24, 8), (32, 2)]


def _build_nc():
    nc = bacc.Bacc("TRN2", target_bir_lowering=False, debug=False,
                   enable_partition_id=False)

    w = nc.declare_dram_parameter("w", [VOCAB * REGION, EMB], mybir.dt.bfloat16, isOutput=False)
    gidx = nc.declare_dram_parameter("gidx", [P, NTILES], mybir.dt.int32, isOutput=False)
    ident = nc.declare_dram_parameter("ident", [P, P + REGION - 1], mybir.dt.bfloat16, isOutput=False)
    out = nc.declare_dram_parameter("out", [TOUT, NTILES * EMB], mybir.dt.bfloat16, isOutput=True)

    from contextlib import ExitStack
    with tile.TileContext(nc) as tc, ExitStack() as ctx:
        const_pool = ctx.enter_context(tc.tile_pool(name="const", bufs=1))
        gpool = ctx.enter_context(tc.tile_pool(name="gather", bufs=4))
        ppool = ctx.enter_context(tc.tile_pool(name="psum", bufs=6, space="PSUM"))
        opool = ctx.enter_context(tc.tile_pool(name="out", bufs=6))

        gidx_sb = const_pool.tile([P, NTILES], mybir.dt.int32)
        idp_sb = const_pool.tile([P, P + REGION - 1], mybir.dt.bfloat16)
        nc.sync.dma_start(gidx_sb[:], gidx.ap())
        nc.scalar.dma_start(idp_sb[:], ident.ap())

        for c0, ng in GROUPS:
            gsb = gpool.tile([P, GRP * BLK], mybir.dt.bfloat16, tag="g")
            gv = gsb[:].rearrange("p (u j e) -> p u j e", u=GRP, j=REGION)
            # one indirect DMA per tile: the HW consumes exactly one index
            # per dest partition and streams 1792B from that base
            for u in range(ng):
                nc.gpsimd.indirect_dma_start(
                    out=gsb[:, u * BLK: (u + 1) * BLK],
                    out_offset=None,
                    in_=w.ap(),
                    in_offset=bass.IndirectOffsetOnAxis(
                        ap=gidx_sb[:, c0 + u: c0 + u + 1], axis=0),
                )
            for h0 in range(0, ng, HALF):
                nu = min(HALF, ng - h0)
                psum = ppool.tile([P, HALF * EMB], mybir.dt.float32, tag="ps")
                for j in range(REGION):
                    nc.tensor.matmul(
                        out=psum[:, : nu * EMB],
                        lhsT=idp_sb[:, j: j + P],
                        rhs=gv[:, h0: h0 + nu, j, :],
                        start=(j == 0),
                        stop=(j == REGION - 1),
                    )
                o = opool.tile([TOUT, HALF * EMB], mybir.dt.bfloat16, tag="o")
                nc.scalar.activation(
                    o[:, : nu * EMB], psum[:TOUT, : nu * EMB],
                    mybir.ActivationFunctionType.Tanh,
                )
                # per-half store: 1KB-per-partition contiguous descriptors,
                # and the final store (-> teardown receipt) fires earlier
                nc.sync.dma_start(
                    out.ap()[:, (c0 + h0) * EMB: (c0 + h0 + nu) * EMB],
                    o[:, : nu * EMB])
    nc.compile()
    return nc


def _host_prep(seq, W):
    s = seq.reshape(B, L)
    w_bf16 = np.ascontiguousarray(W.astype(ml_dtypes.bfloat16))
    ident = np.zeros((P, P + REGION - 1), ml_dtypes.bfloat16)
    ident[np.arange(P), np.arange(P)] = 1

    in_maps = []
    for c in range(NCORES):
        gidx_r = np.zeros((P, NTILES), np.int32)
        for col, (sq, k) in enumerate(TILE_ORDER):
            b = c * SEQ_PER_CORE + sq
            v = k * TOUT - RADIUS + np.arange(P)
            tok = np.where((v >= 0) & (v < L), s[b, np.clip(v, 0, L - 1)], 0)
            gidx_r[:, col] = tok.astype(np.int32) * REGION
        in_maps.append({
            "w": w_bf16,
            "gidx": gidx_r,
            "ident": ident,
        })
    return in_maps


_NC_CACHE = None


def run(seq, W, trace=False, **spmd_kwargs):
    global _NC_CACHE
    if _NC_CACHE is None:
        _NC_CACHE = _build_nc()
    nc = _NC_CACHE
    seq = np.asarray(seq)
    in_maps = _host_prep(seq, np.asarray(W))
    res = run_bass_kernel_spmd(
        nc, in_maps, core_ids=list(range(NCORES)), trace=trace, **spmd_kwargs
    )
    full = np.empty((B, L, EMB), np.float32)
    for c in range(NCORES):
        r = np.asarray(res.results[c]["out"]).astype(np.float32)
        r = r.reshape(TOUT, NTILES, EMB)
        for col, (sq, k) in enumerate(TILE_ORDER):
            b = c * SEQ_PER_CORE + sq
            q0 = k * TOUT
            nrows = min(TOUT, L - q0)
            full[b, q0: q0 + nrows] = r[:nrows, col]
    mask = (seq.reshape(B, L) != 0).astype(np.float32)
    full *= mask[:, :, None]
    return full[:, :, None, :], res


def kernel(seq, W):
    out, _ = run(np.asarray(seq), np.asarray(W))
    return out
